# revision 22
# baseline (speedup 1.0000x reference)
"""DeepCoevolve on Trainium2 (Bass/Tile), 8 NeuronCores.

Strategy (v3)
-------------
Host schedules events into wavefront levels (depth ~4), packs disjoint
components onto 8 cores, renames scatter targets so each step writes a
contiguous column block; only the chained prefix of each step needs an
on-device gather (ap_gather on GPSIMD) -- everything else is pre-gathered
on the host into an fp16 staging buffer.

Device pipeline:
  . all matmuls fp16 x fp16 -> fp32 PSUM
  . per-half GRU biases enter via K=2 selector bias-matmuls (tensor
    engine is cheap) so every ACT runs once, full width
  . tail steps use ONE combined u+v gather into fp32 scratch, then two
    DVE casts into the fp16 staging buffer; a compact per-core init
    region replaces the full embedding-table DMA
  . staging layout is step-interleaved [u_step | v_step] so hcat is
    contiguous and each step's MLP chunk fires right after its cast
  . the device ships raw (dot, mlp logit) per event; the host applies
    -log(softplus(.)+1e-10) and sigmoid (O(n) postprocess)
  . input DMAs ordered so the first step's weights+operands land first;
    outputs for the big region ship mid-kernel
"""

import numpy as np
from contextlib import ExitStack

E = 128
NCORES = 8
LANE = 16        # ap_gather index granularity
MAXB = 256       # max events per step (2B <= 512 f32 = one PSUM bank)

_CACHE = {}
LAST_EXEC_NS = None
TRACE = False


def _round16(x):
    return max(LANE, (int(x) + LANE - 1) // LANE * LANE)


class _Schedule:
    pass


# ----------------------------------------------------------------------------
# host-side scheduling
# ----------------------------------------------------------------------------

def _build_schedule(uid, iid):
    """Wavefront + component schedule. Pure numpy/python, deterministic."""
    uid = np.asarray(uid, np.int64)
    iid = np.asarray(iid, np.int64)
    nev = len(uid)

    # --- wavefront levels ---------------------------------------------------
    lvl = np.zeros(nev, np.int32)
    last_u, last_i = {}, {}
    parent = list(range(nev))

    def find(x):
        while parent[x] != x:
            parent[x] = parent[parent[x]]
            x = parent[x]
        return x

    def union(a, b):
        ra, rb = find(a), find(b)
        if ra != rb:
            parent[ra] = rb

    for e in range(nev):
        l = 0
        a = last_u.get(uid[e])
        if a is not None:
            l = lvl[a] + 1
            union(e, a)
        b = last_i.get(iid[e])
        if b is not None:
            l = max(l, lvl[b] + 1)
            union(e, b)
        lvl[e] = l
        last_u[uid[e]] = e
        last_i[iid[e]] = e

    nlev = int(lvl.max()) + 1

    # --- components -> cores ------------------------------------------------
    comps = {}
    for e in range(nev):
        comps.setdefault(find(e), []).append(e)
    comp_list = sorted(comps.values(), key=len, reverse=True)
    core_events = [[] for _ in range(NCORES)]
    core_tot = [0] * NCORES
    for c in comp_list:
        k = min(range(NCORES), key=lambda i: core_tot[i])
        core_events[k].extend(c)
        core_tot[k] += len(c)

    chained_u = np.zeros(nev, bool)
    chained_v = np.zeros(nev, bool)
    seen_u, seen_i = set(), set()
    for e in range(nev):
        chained_u[e] = uid[e] in seen_u
        chained_v[e] = iid[e] in seen_i
        seen_u.add(uid[e])
        seen_i.add(iid[e])

    queues = [[[] for _ in range(nlev)] for _ in range(NCORES)]
    for k in range(NCORES):
        for e in sorted(core_events[k]):
            queues[k][lvl[e]].append(e)
    for k in range(NCORES):
        for l in range(nlev):
            queues[k][l].sort(
                key=lambda e: (not chained_u[e], not chained_v[e], e))

    # --- step structure (shared by all cores) -------------------------------
    lev_sizes = [_round16(max(len(queues[k][l]) for k in range(NCORES)))
                 for l in range(nlev)]
    steps = []              # [level, B, off]
    off = 0
    for l, m in enumerate(lev_sizes):
        rem = m
        while rem > 0:
            b = min(MAXB, rem)
            steps.append([l, b, off])
            off += b
            rem -= b
    ne = off

    # per-slot staging columns (step-interleaved [u_step | v_step] layout)
    ucol = np.zeros(ne, np.int64)
    vcol = np.zeros(ne, np.int64)
    for (l, b, s_off) in steps:
        for j in range(b):
            ucol[s_off + j] = 2 * s_off + j
            vcol[s_off + j] = 2 * s_off + b + j

    # --- per-core slot fill -------------------------------------------------
    base = 1 << 20
    u_src = np.zeros((NCORES, ne), np.int64)
    i_src = np.zeros((NCORES, ne), np.int64)
    gid = np.full((NCORES, ne), -1, np.int32)
    u_init = [[] for _ in range(NCORES)]
    i_init = [[] for _ in range(NCORES)]
    u_chain_n = np.zeros((NCORES, len(steps)), np.int32)
    v_chain_n = np.zeros((NCORES, len(steps)), np.int32)
    vbase = [base + 2 * s_off for (_, _, s_off) in steps]

    for k in range(NCORES):
        col_u, col_i = {}, {}
        last_su, last_si = {}, {}
        qpos = [0] * nlev
        for s, (l, b, s_off) in enumerate(steps):
            q = queues[k][l]
            take = min(b, len(q) - qpos[l])
            for j in range(take):
                e = q[qpos[l] + j]
                slot = s_off + j
                u, i = uid[e], iid[e]
                if u in last_su:
                    u_src[k, slot] = last_su[u]
                    u_chain_n[k, s] = j + 1
                else:
                    c = col_u.setdefault(u, len(col_u))
                    if c == len(u_init[k]):
                        u_init[k].append(u)
                    u_src[k, slot] = c
                if i in last_si:
                    i_src[k, slot] = last_si[i]
                    v_chain_n[k, s] = j + 1
                else:
                    c = col_i.setdefault(i, len(col_i))
                    if c == len(i_init[k]):
                        i_init[k].append(i)
                    i_src[k, slot] = (base >> 1) + c
                last_su[u] = vbase[s] + j
                last_si[i] = vbase[s] + b + j
                gid[k, slot] = e
            qpos[l] += take
        for s, (l, b, s_off) in enumerate(steps):
            assert u_src[k, s_off:s_off + b].max(initial=0) < vbase[s]
            assert i_src[k, s_off:s_off + b].max(initial=0) < vbase[s]

    ug_n = [0] * len(steps)
    vg_n = [0] * len(steps)
    for s in range(len(steps)):
        mu = int(u_chain_n[:, s].max())
        mv = int(v_chain_n[:, s].max())
        b = steps[s][1]
        ug_n[s] = 0 if mu == 0 else min(b, _round16(mu))
        vg_n[s] = 0 if mv == 0 else min(b, _round16(mv))

    # --- compact init region -----------------------------------------------
    cmap = [dict() for _ in range(NCORES)]   # src id -> compact col
    for k in range(NCORES):
        m = cmap[k]
        for s, (l, b, s_off) in enumerate(steps):
            for j in range(ug_n[s]):
                src = u_src[k, s_off + j]
                if src < base and src not in m:
                    m[src] = len(m)
            for j in range(vg_n[s]):
                src = i_src[k, s_off + j]
                if src < base and src not in m:
                    m[src] = len(m)
    ncompact = _round16(max(1, max(len(m) for m in cmap)))

    def dev_col(k, src):
        if src >= base:
            return ncompact + (src - base)
        return cmap[k].get(src, 0)

    sc = _Schedule()
    sc.nev, sc.ne = nev, ne
    sc.base = base
    sc.ncompact = ncompact
    sc.nvcols = ncompact + 2 * ne
    assert sc.nvcols <= 2 ** 13, sc.nvcols
    sc.u_src, sc.i_src, sc.gid = u_src, i_src, gid
    sc.u_init, sc.i_init = u_init, i_init
    sc.cmap = cmap
    sc.dev_col = dev_col
    sc.ucol, sc.vcol = ucol, vcol

    icol = 0
    gsteps = []   # (l, b, s_off, vb_col, ic, un, vn)
    for s, (l, b, s_off) in enumerate(steps):
        un, vn = ug_n[s], vg_n[s]
        gsteps.append((l, b, s_off, ncompact + 2 * s_off, icol, un, vn))
        icol += ((un + vn) // LANE + 1) // 2 * 2
    sc.steps = gsteps
    sc.nicol = max(2, icol)
    return sc


def _wrap_idx_combined(sc, k):
    """Per-step combined u+v gather indices [128, nicol] int16."""
    out = np.zeros((16, sc.nicol), np.int16)
    for (_, b, off, _, ic, un, vn) in sc.steps:
        if un + vn == 0:
            continue
        idx = np.zeros(un + vn, np.int64)
        for j in range(un):
            idx[j] = sc.dev_col(k, sc.u_src[k, off + j])
        for j in range(vn):
            idx[un + j] = sc.dev_col(k, sc.i_src[k, off + j])
        w = idx.reshape(-1, LANE).T.astype(np.int16)
        out[:, ic:ic + (un + vn) // LANE] = w
    return np.tile(out, (8, 1))


def _prep_shared(inp):
    """Weight + bias-selector stacks shared by all cores (fp16)."""
    f = np.float32
    uwi, uwh = inp["ugru_wi"].astype(f), inp["ugru_wh"].astype(f)
    iwi, iwh = inp["igru_wi"].astype(f), inp["igru_wh"].astype(f)
    t1w, t2w, t3w = inp["t1_w"].astype(f), inp["t2_w"].astype(f), inp["t3_w"].astype(f)

    blocks = []
    for g in (0, 1):                                  # r, z
        s = slice(g * E, (g + 1) * E)
        blocks += [uwi[s].T, uwh[s].T, iwi[s].T, iwh[s].T]
    s = slice(2 * E, 3 * E)
    blocks += [uwi[s].T, iwi[s].T]                    # inn (applied to x)
    blocks += [uwh[s].T, iwh[s].T]                    # hn  (applied to h)
    blocks += [t1w[:, :E].T, t1w[:, E:].T, t2w.T]     # 128,128,32 cols
    wstack = np.concatenate(blocks, axis=1)
    extra = np.zeros((E, 2), f)
    extra[:32, 0] = t3w[0]
    extra[:, 1] = 1.0
    wstack = np.concatenate([wstack, extra], axis=1)  # t3 col, ones col

    ub_i, ub_h = inp["ugru_bi"].astype(f), inp["ugru_bh"].astype(f)
    ib_i, ib_h = inp["igru_bi"].astype(f), inp["igru_bh"].astype(f)
    # K=2 selector bias pairs [2, 4E]: r, z, inn, hn
    bst = np.zeros((2, 4 * E), f)
    bst[0, 0:E] = ub_i[0:E] + ub_h[0:E]
    bst[1, 0:E] = ib_i[0:E] + ib_h[0:E]
    bst[0, E:2 * E] = ub_i[E:2 * E] + ub_h[E:2 * E]
    bst[1, E:2 * E] = ib_i[E:2 * E] + ib_h[E:2 * E]
    bst[0, 2 * E:3 * E] = ub_i[2 * E:]
    bst[1, 2 * E:3 * E] = ib_i[2 * E:]
    bst[0, 3 * E:] = ub_h[2 * E:]
    bst[1, 3 * E:] = ib_h[2 * E:]

    # ACT-bias columns [E, 4]: t1_b, t2_b, t3_b, zero
    bm = np.zeros((E, 4), f)
    bm[:, 0] = inp["t1_b"].astype(f)
    bm[:32, 1] = inp["t2_b"].astype(f)
    bm[0, 2] = inp["t3_b"].astype(f)[0]
    return (wstack.astype(np.float16), bst.astype(np.float16), bm)


def _sel_array(sc):
    sel = np.zeros((2, 2 * sc.ne), np.float16)
    for (_, b, off, _, _, _, _) in sc.steps:
        sel[0, 2 * off: 2 * off + b] = 1.0
        sel[1, 2 * off + b: 2 * off + 2 * b] = 1.0
    return sel


def _core_inputs(inp, sc, k):
    """Per-core fp16 staging prefill, compact vbuf init, gather indices."""
    f = np.float32
    uemb = inp["user_emb"]
    iemb = inp["item_emb"]

    nu = len(sc.u_init[k])
    ni = len(sc.i_init[k])
    uvals = uemb[np.asarray(sc.u_init[k], np.int64)].T.astype(f) if nu else np.zeros((E, 0), f)
    ivals = iemb[np.asarray(sc.i_init[k], np.int64)].T.astype(f) if ni else np.zeros((E, 0), f)

    def init_val(src):
        if src >= (sc.base >> 1):
            c = src - (sc.base >> 1)
            return ivals[:, c] if c < ni else np.zeros(E, f)
        return uvals[:, src] if src < nu else np.zeros(E, f)

    hs16 = np.zeros((E, 2 * sc.ne), np.float16)
    for slot in range(sc.ne):
        us = sc.u_src[k, slot]
        if us < sc.base:
            hs16[:, sc.ucol[slot]] = init_val(us).astype(np.float16)
        vs = sc.i_src[k, slot]
        if vs < sc.base:
            hs16[:, sc.vcol[slot]] = init_val(vs).astype(np.float16)

    vbinit = np.zeros((E, sc.ncompact), f)
    for src, c in sc.cmap[k].items():
        vbinit[:, c] = init_val(src)

    gidx = _wrap_idx_combined(sc, k)
    return hs16, vbinit, gidx


# ----------------------------------------------------------------------------
# pure-numpy model of the scheduled computation (validation / debugging)
# ----------------------------------------------------------------------------

def _numpy_model(inp, sc):
    f16 = np.float16
    wstack, bst, bm = _prep_shared(inp)
    ws = wstack.astype(np.float32)
    bs = bst.astype(np.float32)
    ne = sc.ne
    out = np.zeros((sc.nev, 2), np.float32)

    def blk(i):
        return ws[:, i * E:(i + 1) * E]

    for k in range(NCORES):
        hs16, vbinit, _ = _core_inputs(inp, sc, k)
        vb = np.zeros((E, sc.nvcols), np.float32)
        vb[:, :sc.ncompact] = vbinit
        hs = hs16.astype(np.float32)
        dotv = np.zeros(ne, np.float32)
        logitv = np.zeros(ne, np.float32)
        for (l, b, off, vbc, _, un, vn) in sc.steps:
            for j in range(un):
                c = sc.dev_col(k, sc.u_src[k, off + j])
                hs[:, 2 * off + j] = vb[:, c].astype(f16).astype(np.float32)
            for j in range(vn):
                c = sc.dev_col(k, sc.i_src[k, off + j])
                hs[:, 2 * off + b + j] = vb[:, c].astype(f16).astype(np.float32)
            ug = hs[:, 2 * off:2 * off + b]
            vg = hs[:, 2 * off + b:2 * off + 2 * b]
            bsel = np.zeros((E, 2 * b), np.float32)
            pr = np.concatenate([np.tile(bs[0:1, 0:E].T, b),
                                 np.tile(bs[1:2, 0:E].T, b)], 1)
            pr[:, :b] += blk(0).T @ vg + blk(1).T @ ug
            pr[:, b:] += blk(2).T @ ug + blk(3).T @ vg
            pz = np.concatenate([np.tile(bs[0:1, E:2 * E].T, b),
                                 np.tile(bs[1:2, E:2 * E].T, b)], 1)
            pz[:, :b] += blk(4).T @ vg + blk(5).T @ ug
            pz[:, b:] += blk(6).T @ ug + blk(7).T @ vg
            pinn = np.concatenate([np.tile(bs[0:1, 2 * E:3 * E].T, b),
                                   np.tile(bs[1:2, 2 * E:3 * E].T, b)], 1)
            pinn[:, :b] += blk(8).T @ vg
            pinn[:, b:] += blk(9).T @ ug
            phn = np.concatenate([np.tile(bs[0:1, 3 * E:].T, b),
                                  np.tile(bs[1:2, 3 * E:].T, b)], 1)
            phn[:, :b] += blk(10).T @ ug
            phn[:, b:] += blk(11).T @ vg
            r = (1.0 / (1.0 + np.exp(-pr))).astype(f16).astype(np.float32)
            z = (1.0 / (1.0 + np.exp(-pz))).astype(f16).astype(np.float32)
            n = np.tanh(phn * r + pinn).astype(f16).astype(np.float32)
            hcat = np.concatenate([ug, vg], axis=1)
            d = (hcat - n).astype(f16).astype(np.float32)
            m = (z * d).astype(f16).astype(np.float32)
            vb[:, vbc:vbc + 2 * b] = n + m
            # per-step MLP
            t1a = ws[:, 12 * E:13 * E]
            t1b = ws[:, 13 * E:14 * E]
            t2 = ws[:, 14 * E:14 * E + 32]
            t3 = ws[:32, 14 * E + 32]
            h1 = np.maximum(t1a.T @ ug + t1b.T @ vg + bm[:, 0:1], 0.0)
            h1 = h1.astype(f16).astype(np.float32)
            h2 = np.maximum(t2.T @ h1 + bm[:32, 1:2], 0.0)
            h2 = h2.astype(f16).astype(np.float32)
            logitv[off:off + b] = t3 @ h2 + bm[0, 2]
            uvm = (ug * vg).astype(f16).astype(np.float32)
            dotv[off:off + b] = uvm.sum(axis=0)
        mask = sc.gid[k] >= 0
        g = sc.gid[k][mask]
        out[g, 0] = dotv[mask]
        out[g, 1] = logitv[mask]
    return out


# ----------------------------------------------------------------------------
# device program
# ----------------------------------------------------------------------------

def _build_program(sc):
    import concourse.bass as bass
    import concourse.tile as tile
    from concourse import bacc, mybir
    from concourse.tile_rust import add_dep_helper

    f32 = mybir.dt.float32
    f16 = mybir.dt.float16
    i16 = mybir.dt.int16
    ne = sc.ne
    W = 14 * E + 32 + 2    # wstack cols
    W3 = 14 * E + 32       # t3 col
    WON = W3 + 1           # ones col
    AF = mybir.ActivationFunctionType
    OP = mybir.AluOpType

    nc = bacc.Bacc("TRN2", target_bir_lowering=False, debug=False)
    d_w = nc.dram_tensor("wstack", [E, W], f16, kind="ExternalInput").ap()
    d_bs = nc.dram_tensor("bstack", [2, 4 * E], f16, kind="ExternalInput").ap()
    d_bm = nc.dram_tensor("bmisc", [E, 4], f32, kind="ExternalInput").ap()
    d_sel = nc.dram_tensor("sel", [2, 2 * ne], f16, kind="ExternalInput").ap()
    d_hs = nc.dram_tensor("hsinit", [E, 2 * ne], f16, kind="ExternalInput").ap()
    d_gi = nc.dram_tensor("gidx", [E, sc.nicol], i16, kind="ExternalInput").ap()
    d_vb = nc.dram_tensor("vbinit", [E, sc.ncompact], f32, kind="ExternalInput").ap()
    d_out = nc.dram_tensor("outdl", [1, 2 * ne], f32, kind="ExternalOutput").ap()

    b0 = sc.steps[0][1]    # first-step width for the priority DMA slice

    with tile.TileContext(nc) as tc, ExitStack() as ctx:
        const = ctx.enter_context(tc.tile_pool(name="const", bufs=1))
        psum = ctx.enter_context(tc.tile_pool(name="psum", bufs=2, space="PSUM"))
        work = ctx.enter_context(tc.tile_pool(name="work", bufs=2))

        # dummy gather first: pulls the ext-isa GPSIMD library into IRAM
        # while the input DMAs stream in parallel.
        warm = const.tile([E, 16], f32)
        nc.vector.memset(warm[:], 0.0)
        warmi = const.tile([E, 2], i16)
        nc.vector.memset(warmi[:].bitcast(f32), 0.0)
        warmo = const.tile([E, 16], f32)
        nc.gpsimd.ap_gather(warmo[:], warm[:], warmi[:, 0:1],
                            channels=E, num_elems=16, d=1, num_idxs=16)

        # warm the gpsimd elementwise ucode path too (used for offloaded TTs)
        nc.gpsimd.tensor_tensor(out=warm[:], in0=warm[:], in1=warm[:], op=OP.add)

        # input DMAs, priority order: step-0 working set first
        wsb = const.tile([E, W], f16)
        nc.sync.dma_start(wsb[:, 0:12 * E], d_w[:, 0:12 * E])
        bsb = const.tile([2, 4 * E], f16)
        nc.sync.dma_start(bsb[:], d_bs[:])
        selsb = const.tile([2, 2 * ne], f16)
        nc.sync.dma_start(selsb[:], d_sel[:])
        hs = const.tile([E, 2 * ne], f16)
        nc.sync.dma_start(hs[:, 0:2 * b0], d_hs[:, 0:2 * b0])
        bmsb = const.tile([E, 4], f32)
        nc.sync.dma_start(bmsb[:], d_bm[:])
        nc.sync.dma_start(hs[:, 2 * b0:], d_hs[:, 2 * b0:])
        nc.sync.dma_start(wsb[:, 12 * E:], d_w[:, 12 * E:])
        gidx = const.tile([E, sc.nicol], i16)
        nc.sync.dma_start(gidx[:], d_gi[:])
        vbuf = const.tile([E, sc.nvcols], f32)
        nc.sync.dma_start(vbuf[:, :sc.ncompact], d_vb[:])
        scr = const.tile([E, 64], f32)
        outsb = const.tile([1, 2 * ne], f32)   # [dot | logit]

        def mm(out_ap, wcol, rhs_ap, start, stop):
            nc.tensor.matmul(
                out_ap,
                lhsT=wsb[:, wcol * E:(wcol + 1) * E],
                rhs=rhs_ap,
                start=start, stop=stop, skip_group_check=True,
            )

        def gru_mms(pt4, ug, vg, off, b):
            pr, pz, pinn, phn = pt4
            selb = selsb[:, 2 * off:2 * off + 2 * b]
            for gi, pt in enumerate((pr, pz, pinn, phn)):
                nc.tensor.matmul(
                    pt[:, 0:2 * b], lhsT=bsb[:, gi * E:(gi + 1) * E],
                    rhs=selb, start=True, stop=False, skip_group_check=True)
            # r first (critical path), then hn, inn, z
            mm(pr[:, 0:b], 0, vg, False, False)
            mm(pr[:, 0:b], 1, ug, False, True)
            mm(pr[:, b:2 * b], 2, ug, False, False)
            mm(pr[:, b:2 * b], 3, vg, False, True)
            mm(phn[:, 0:b], 10, ug, False, True)
            mm(phn[:, b:2 * b], 11, vg, False, True)
            mm(pinn[:, 0:b], 8, vg, False, True)
            mm(pinn[:, b:2 * b], 9, ug, False, True)
            mm(pz[:, 0:b], 4, vg, False, False)
            mm(pz[:, 0:b], 5, ug, False, True)
            mm(pz[:, b:2 * b], 6, ug, False, False)
            mm(pz[:, b:2 * b], 7, vg, False, True)

        def gru_tail(pt4, step, wb_list, eng=None):
            (l, b, off, vbc, ic, un, vn) = step
            pr, pz, pinn, phn = pt4
            ve = eng if eng is not None else nc.vector
            r = work.tile([E, 2 * b], f16, tag="r")
            z = work.tile([E, 2 * b], f16, tag="z")
            nfn = work.tile([E, 2 * b], f16, tag="nfn")
            tmp = work.tile([E, 2 * b], f32, tag="tmp")
            d16 = work.tile([E, 2 * b], f16, tag="d16")
            nc.scalar.activation(r[:], pr[:], AF.Sigmoid)
            # PSUM-reading ops must stay on the vector engine
            nc.vector.tensor_tensor(out=tmp[:], in0=phn[:], in1=r[:], op=OP.mult)
            nc.vector.tensor_tensor(out=tmp[:], in0=tmp[:], in1=pinn[:], op=OP.add)
            nc.scalar.activation(nfn[:], tmp[:], AF.Tanh)
            nc.scalar.activation(z[:], pz[:], AF.Sigmoid)
            hcat = hs[:, 2 * off:2 * off + 2 * b]
            ve.tensor_tensor(out=d16[:], in0=hcat, in1=nfn[:], op=OP.subtract)
            ve.tensor_tensor(out=d16[:], in0=z[:], in1=d16[:], op=OP.mult)
            wb = ve.tensor_tensor(
                out=vbuf[:, vbc:vbc + 2 * b],
                in0=nfn[:], in1=d16[:], op=OP.add)
            wb_list.append(wb)

        wb_list = []

        def gather_cast(step):
            (l, b, off, vbc, ic, un, vn) = step
            if un + vn == 0:
                return
            g = nc.gpsimd.ap_gather(
                scr[:, 0:un + vn], vbuf[:], gidx[:, ic:ic + (un + vn) // LANE],
                channels=E, num_elems=sc.nvcols, d=1, num_idxs=un + vn)
            for wb in wb_list[-2:]:
                add_dep_helper(g.ins, wb.ins,
                               reason="gather reads prev writeback")
            if un:
                nc.vector.tensor_copy(out=hs[:, 2 * off:2 * off + un],
                                      in_=scr[:, 0:un])
            if vn:
                nc.vector.tensor_copy(out=hs[:, 2 * off + b:2 * off + b + vn],
                                      in_=scr[:, un:un + vn])

        def mlp_chunk(step, uvm_eng=None):
            (l, b, off, vbc, ic, un, vn) = step
            ue = uvm_eng if uvm_eng is not None else nc.vector
            u_c = hs[:, 2 * off:2 * off + b]
            v_c = hs[:, 2 * off + b:2 * off + 2 * b]
            h1p = psum.tile([E, b], f32, tag="pr", name="h1p")
            mm(h1p[:], 12, u_c, True, False)
            mm(h1p[:], 13, v_c, False, True)
            h1 = work.tile([E, b], f16, tag="r", name="h1")
            nc.scalar.activation(h1[:], h1p[:], AF.Relu, bias=bmsb[:, 0:1])
            h2p = psum.tile([32, b], f32, tag="pz", name="h2p")
            nc.tensor.matmul(h2p[:], lhsT=wsb[:, 14 * E:14 * E + 32],
                             rhs=h1[:], start=True, stop=True,
                             skip_group_check=True)
            h2 = work.tile([32, b], f16, tag="z", name="h2")
            nc.scalar.activation(h2[:], h2p[:], AF.Relu, bias=bmsb[:32, 1:2])
            h3p = psum.tile([1, b], f32, tag="pinn", name="h3p")
            nc.tensor.matmul(h3p[:], lhsT=wsb[:32, W3:W3 + 1],
                             rhs=h2[:], start=True, stop=True,
                             skip_group_check=True)
            nc.scalar.activation(outsb[0:1, ne + off:ne + off + b], h3p[:],
                                 AF.Identity, bias=bmsb[0:1, 2:3])
            uvm = work.tile([E, b], f16, tag="nfn", name="uvm")
            ue.tensor_tensor(out=uvm[:], in0=u_c, in1=v_c, op=OP.mult)
            dotp = psum.tile([1, b], f32, tag="phn", name="dotp")
            nc.tensor.matmul(dotp[:], lhsT=wsb[:, WON:WON + 1],
                             rhs=uvm[:], start=True, stop=True,
                             skip_group_check=True)
            nc.vector.tensor_copy(out=outsb[0:1, off:off + b], in_=dotp[:])

        def psum4(b):
            return tuple(psum.tile([E, 2 * b], f32, tag=t, name=f"p_{t}")
                         for t in ("pr", "pz", "pinn", "phn"))

        big = [st for st in sc.steps if st[5] + st[6] == 0]
        tail = [st for st in sc.steps if st[5] + st[6] > 0]
        assert len(big) <= 2, "level-0 region must fit 2 PSUM generations"

        pts = {}
        for st in big:
            (l, b, off) = st[0], st[1], st[2]
            pt4 = psum4(b)
            pts[off] = pt4
            ug = hs[:, 2 * off:2 * off + b]
            vg = hs[:, 2 * off + b:2 * off + 2 * b]
            gru_mms(pt4, ug, vg, off, b)
        for si, st in enumerate(big):
            gru_tail(pts[st[2]], st, wb_list,
                     eng=nc.gpsimd if si == 0 else None)

        if tail:
            gather_cast(tail[0])
        for i, st in enumerate(tail):
            (l, b, off) = st[0], st[1], st[2]
            pt4 = psum4(b)
            ug = hs[:, 2 * off:2 * off + b]
            vg = hs[:, 2 * off + b:2 * off + 2 * b]
            gru_mms(pt4, ug, vg, off, b)
            gru_tail(pt4, st, wb_list)
            # issue the next gather immediately after this writeback so the
            # GPSIMD wake latency hides under the MLP backlog below
            if i + 1 < len(tail):
                gather_cast(tail[i + 1])
            if i < len(big):
                mlp_chunk(big[i], uvm_eng=nc.gpsimd)
            mlp_chunk(st)
            if i == len(tail) - 1:
                for j in range(len(tail), len(big)):
                    mlp_chunk(big[j])
                # ship the big-region outputs while the tail finishes
                big_end = big[-1][2] + big[-1][1] if big else 0
                if big_end:
                    nc.sync.dma_start(d_out[:, 0:big_end], outsb[:, 0:big_end])
                    nc.sync.dma_start(d_out[:, ne:ne + big_end],
                                      outsb[:, ne:ne + big_end])
        if not tail:
            for st in big:
                mlp_chunk(st)
            big_end = 0

        tail_start = big[-1][2] + big[-1][1] if big else 0
        nc.sync.dma_start(d_out[:, tail_start:ne], outsb[:, tail_start:ne])
        nc.sync.dma_start(d_out[:, ne + tail_start:], outsb[:, ne + tail_start:])

    nc.compile()
    return nc


# ----------------------------------------------------------------------------
# entry point
# ----------------------------------------------------------------------------

def kernel(**inputs):
    global LAST_EXEC_NS
    from concourse.bass_utils import run_bass_kernel_spmd

    uid = np.asarray(inputs["user_ids"])
    iid = np.asarray(inputs["item_ids"])
    key = (uid.tobytes(), iid.tobytes())
    if key not in _CACHE:
        sc = _build_schedule(uid, iid)
        nc = _build_program(sc)
        _CACHE[key] = (sc, nc)
    sc, nc = _CACHE[key]

    wstack, bst, bm = _prep_shared(inputs)
    sel = _sel_array(sc)
    in_maps = []
    for k in range(NCORES):
        hs16, vbinit, gidx = _core_inputs(inputs, sc, k)
        in_maps.append({
            "wstack": wstack, "bstack": bst, "bmisc": bm, "sel": sel,
            "hsinit": hs16, "gidx": gidx, "vbinit": vbinit,
        })

    res = run_bass_kernel_spmd(nc, in_maps, list(range(NCORES)), trace=TRACE)
    LAST_EXEC_NS = res.exec_time_ns

    out = np.zeros((sc.nev, 2), np.float32)
    for k in range(NCORES):
        mask = sc.gid[k] >= 0
        g = sc.gid[k][mask]
        o = res.results[k]["outdl"][0]
        dot = o[:sc.ne][mask].astype(np.float64)
        logit = o[sc.ne:][mask].astype(np.float64)
        sp = np.logaddexp(0.0, dot)
        out[g, 0] = (-np.log(sp + 1e-10)).astype(np.float32)
        out[g, 1] = (1.0 / (1.0 + np.exp(-logit))).astype(np.float32)
    return out


# revision 25
# speedup vs baseline: 1.6199x; 1.6199x over previous
"""DeepCoevolve on Trainium2 (Bass/Tile), 8 NeuronCores.

Strategy (v3)
-------------
Host schedules events into wavefront levels (depth ~4), packs disjoint
components onto 8 cores, renames scatter targets so each step writes a
contiguous column block; only the chained prefix of each step needs an
on-device gather (ap_gather on GPSIMD) -- everything else is pre-gathered
on the host into an fp16 staging buffer.

Device pipeline:
  . all matmuls fp16 x fp16 -> fp32 PSUM
  . per-half GRU biases enter via K=2 selector bias-matmuls (tensor
    engine is cheap) so every ACT runs once, full width
  . tail steps use ONE combined u+v gather into fp32 scratch, then two
    DVE casts into the fp16 staging buffer; a compact per-core init
    region replaces the full embedding-table DMA
  . staging layout is step-interleaved [u_step | v_step] so hcat is
    contiguous and each step's MLP chunk fires right after its cast
  . the device ships raw (dot, mlp logit) per event; the host applies
    -log(softplus(.)+1e-10) and sigmoid (O(n) postprocess)
  . input DMAs ordered so the first step's weights+operands land first;
    outputs for the big region ship mid-kernel
"""

import numpy as np
from contextlib import ExitStack

E = 128
NCORES = 8
LANE = 16        # ap_gather index granularity
MAXB = 256       # max events per step (2B <= 512 f32 = one PSUM bank)

_CACHE = {}
LAST_EXEC_NS = None
TRACE = False


def _round16(x):
    return max(LANE, (int(x) + LANE - 1) // LANE * LANE)


class _Schedule:
    pass


# ----------------------------------------------------------------------------
# host-side scheduling
# ----------------------------------------------------------------------------

def _build_schedule(uid, iid):
    """Wavefront + component schedule. Pure numpy/python, deterministic."""
    uid = np.asarray(uid, np.int64)
    iid = np.asarray(iid, np.int64)
    nev = len(uid)

    # --- wavefront levels ---------------------------------------------------
    lvl = np.zeros(nev, np.int32)
    last_u, last_i = {}, {}
    parent = list(range(nev))

    def find(x):
        while parent[x] != x:
            parent[x] = parent[parent[x]]
            x = parent[x]
        return x

    def union(a, b):
        ra, rb = find(a), find(b)
        if ra != rb:
            parent[ra] = rb

    for e in range(nev):
        l = 0
        a = last_u.get(uid[e])
        if a is not None:
            l = lvl[a] + 1
            union(e, a)
        b = last_i.get(iid[e])
        if b is not None:
            l = max(l, lvl[b] + 1)
            union(e, b)
        lvl[e] = l
        last_u[uid[e]] = e
        last_i[iid[e]] = e

    nlev = int(lvl.max()) + 1

    # --- components -> cores ------------------------------------------------
    comps = {}
    for e in range(nev):
        comps.setdefault(find(e), []).append(e)
    comp_list = sorted(comps.values(), key=len, reverse=True)
    core_events = [[] for _ in range(NCORES)]
    core_tot = [0] * NCORES
    for c in comp_list:
        k = min(range(NCORES), key=lambda i: core_tot[i])
        core_events[k].extend(c)
        core_tot[k] += len(c)

    chained_u = np.zeros(nev, bool)
    chained_v = np.zeros(nev, bool)
    seen_u, seen_i = set(), set()
    for e in range(nev):
        chained_u[e] = uid[e] in seen_u
        chained_v[e] = iid[e] in seen_i
        seen_u.add(uid[e])
        seen_i.add(iid[e])

    queues = [[[] for _ in range(nlev)] for _ in range(NCORES)]
    for k in range(NCORES):
        for e in sorted(core_events[k]):
            queues[k][lvl[e]].append(e)
    for k in range(NCORES):
        for l in range(nlev):
            queues[k][l].sort(
                key=lambda e: (not chained_u[e], not chained_v[e], e))

    # --- step structure (shared by all cores) -------------------------------
    lev_sizes = [_round16(max(len(queues[k][l]) for k in range(NCORES)))
                 for l in range(nlev)]
    steps = []              # [level, B, off]
    off = 0
    for l, m in enumerate(lev_sizes):
        rem = m
        while rem > 0:
            b = min(MAXB, rem)
            steps.append([l, b, off])
            off += b
            rem -= b
    ne = off

    # per-slot staging columns (step-interleaved [u_step | v_step] layout)
    ucol = np.zeros(ne, np.int64)
    vcol = np.zeros(ne, np.int64)
    for (l, b, s_off) in steps:
        for j in range(b):
            ucol[s_off + j] = 2 * s_off + j
            vcol[s_off + j] = 2 * s_off + b + j

    # --- per-core slot fill -------------------------------------------------
    base = 1 << 20
    u_src = np.zeros((NCORES, ne), np.int64)
    i_src = np.zeros((NCORES, ne), np.int64)
    gid = np.full((NCORES, ne), -1, np.int32)
    u_init = [[] for _ in range(NCORES)]
    i_init = [[] for _ in range(NCORES)]
    u_chain_n = np.zeros((NCORES, len(steps)), np.int32)
    v_chain_n = np.zeros((NCORES, len(steps)), np.int32)
    vbase = [base + 2 * s_off for (_, _, s_off) in steps]

    for k in range(NCORES):
        col_u, col_i = {}, {}
        last_su, last_si = {}, {}
        qpos = [0] * nlev
        for s, (l, b, s_off) in enumerate(steps):
            q = queues[k][l]
            take = min(b, len(q) - qpos[l])
            for j in range(take):
                e = q[qpos[l] + j]
                slot = s_off + j
                u, i = uid[e], iid[e]
                if u in last_su:
                    u_src[k, slot] = last_su[u]
                    u_chain_n[k, s] = j + 1
                else:
                    c = col_u.setdefault(u, len(col_u))
                    if c == len(u_init[k]):
                        u_init[k].append(u)
                    u_src[k, slot] = c
                if i in last_si:
                    i_src[k, slot] = last_si[i]
                    v_chain_n[k, s] = j + 1
                else:
                    c = col_i.setdefault(i, len(col_i))
                    if c == len(i_init[k]):
                        i_init[k].append(i)
                    i_src[k, slot] = (base >> 1) + c
                last_su[u] = vbase[s] + j
                last_si[i] = vbase[s] + b + j
                gid[k, slot] = e
            qpos[l] += take
        for s, (l, b, s_off) in enumerate(steps):
            assert u_src[k, s_off:s_off + b].max(initial=0) < vbase[s]
            assert i_src[k, s_off:s_off + b].max(initial=0) < vbase[s]

    ug_n = [0] * len(steps)
    vg_n = [0] * len(steps)
    for s in range(len(steps)):
        mu = int(u_chain_n[:, s].max())
        mv = int(v_chain_n[:, s].max())
        b = steps[s][1]
        ug_n[s] = 0 if mu == 0 else min(b, _round16(mu))
        vg_n[s] = 0 if mv == 0 else min(b, _round16(mv))

    # --- compact init region -----------------------------------------------
    cmap = [dict() for _ in range(NCORES)]   # src id -> compact col
    for k in range(NCORES):
        m = cmap[k]
        for s, (l, b, s_off) in enumerate(steps):
            for j in range(ug_n[s]):
                src = u_src[k, s_off + j]
                if src < base and src not in m:
                    m[src] = len(m)
            for j in range(vg_n[s]):
                src = i_src[k, s_off + j]
                if src < base and src not in m:
                    m[src] = len(m)
    ncompact = _round16(max(1, max(len(m) for m in cmap)))

    def dev_col(k, src):
        if src >= base:
            return ncompact + (src - base)
        return cmap[k].get(src, 0)

    sc = _Schedule()
    sc.nev, sc.ne = nev, ne
    sc.base = base
    sc.ncompact = ncompact
    sc.nvcols = ncompact + 2 * ne
    assert sc.nvcols <= 2 ** 13, sc.nvcols
    sc.u_src, sc.i_src, sc.gid = u_src, i_src, gid
    sc.u_init, sc.i_init = u_init, i_init
    sc.cmap = cmap
    sc.dev_col = dev_col
    sc.ucol, sc.vcol = ucol, vcol

    icol = 0
    gsteps = []   # (l, b, s_off, vb_col, ic, un, vn)
    for s, (l, b, s_off) in enumerate(steps):
        un, vn = ug_n[s], vg_n[s]
        gsteps.append((l, b, s_off, ncompact + 2 * s_off, icol, un, vn))
        icol += ((un + vn) // LANE + 1) // 2 * 2
    sc.steps = gsteps
    sc.nicol = max(2, icol)
    return sc


def _wrap_idx_combined(sc, k):
    """Per-step combined u+v gather indices [128, nicol] int16."""
    out = np.zeros((16, sc.nicol), np.int16)
    for (_, b, off, _, ic, un, vn) in sc.steps:
        if un + vn == 0:
            continue
        idx = np.zeros(un + vn, np.int64)
        for j in range(un):
            idx[j] = sc.dev_col(k, sc.u_src[k, off + j])
        for j in range(vn):
            idx[un + j] = sc.dev_col(k, sc.i_src[k, off + j])
        w = idx.reshape(-1, LANE).T.astype(np.int16)
        out[:, ic:ic + (un + vn) // LANE] = w
    return np.tile(out, (8, 1))


def _prep_shared(inp):
    """Weight + bias-selector stacks shared by all cores (fp16)."""
    f = np.float32
    uwi, uwh = inp["ugru_wi"].astype(f), inp["ugru_wh"].astype(f)
    iwi, iwh = inp["igru_wi"].astype(f), inp["igru_wh"].astype(f)
    t1w, t2w, t3w = inp["t1_w"].astype(f), inp["t2_w"].astype(f), inp["t3_w"].astype(f)

    blocks = []
    for g in (0, 1):                                  # r, z
        s = slice(g * E, (g + 1) * E)
        blocks += [uwi[s].T, uwh[s].T, iwi[s].T, iwh[s].T]
    s = slice(2 * E, 3 * E)
    blocks += [uwi[s].T, iwi[s].T]                    # inn (applied to x)
    blocks += [uwh[s].T, iwh[s].T]                    # hn  (applied to h)
    blocks += [t1w[:, :E].T, t1w[:, E:].T, t2w.T]     # 128,128,32 cols
    wstack = np.concatenate(blocks, axis=1)
    extra = np.zeros((E, 2), f)
    extra[:32, 0] = t3w[0]
    extra[:, 1] = 1.0
    wstack = np.concatenate([wstack, extra], axis=1)  # t3 col, ones col

    ub_i, ub_h = inp["ugru_bi"].astype(f), inp["ugru_bh"].astype(f)
    ib_i, ib_h = inp["igru_bi"].astype(f), inp["igru_bh"].astype(f)
    # K=2 selector bias pairs [2, 4E]: r, z, inn, hn
    bst = np.zeros((2, 4 * E), f)
    bst[0, 0:E] = ub_i[0:E] + ub_h[0:E]
    bst[1, 0:E] = ib_i[0:E] + ib_h[0:E]
    bst[0, E:2 * E] = ub_i[E:2 * E] + ub_h[E:2 * E]
    bst[1, E:2 * E] = ib_i[E:2 * E] + ib_h[E:2 * E]
    bst[0, 2 * E:3 * E] = ub_i[2 * E:]
    bst[1, 2 * E:3 * E] = ib_i[2 * E:]
    bst[0, 3 * E:] = ub_h[2 * E:]
    bst[1, 3 * E:] = ib_h[2 * E:]

    # ACT-bias columns [E, 4]: t1_b, t2_b, t3_b, zero
    bm = np.zeros((E, 4), f)
    bm[:, 0] = inp["t1_b"].astype(f)
    bm[:32, 1] = inp["t2_b"].astype(f)
    bm[0, 2] = inp["t3_b"].astype(f)[0]
    return (wstack.astype(np.float16), bst.astype(np.float16), bm)


def _sel_array(sc):
    sel = np.zeros((2, 2 * sc.ne), np.float16)
    for (_, b, off, _, _, _, _) in sc.steps:
        sel[0, 2 * off: 2 * off + b] = 1.0
        sel[1, 2 * off + b: 2 * off + 2 * b] = 1.0
    return sel


def _core_inputs(inp, sc, k):
    """Per-core fp16 staging prefill, compact vbuf init, gather indices."""
    f = np.float32
    uemb = inp["user_emb"]
    iemb = inp["item_emb"]

    nu = len(sc.u_init[k])
    ni = len(sc.i_init[k])
    uvals = uemb[np.asarray(sc.u_init[k], np.int64)].T.astype(f) if nu else np.zeros((E, 0), f)
    ivals = iemb[np.asarray(sc.i_init[k], np.int64)].T.astype(f) if ni else np.zeros((E, 0), f)

    def init_val(src):
        if src >= (sc.base >> 1):
            c = src - (sc.base >> 1)
            return ivals[:, c] if c < ni else np.zeros(E, f)
        return uvals[:, src] if src < nu else np.zeros(E, f)

    hs16 = np.zeros((E, 2 * sc.ne), np.float16)
    for slot in range(sc.ne):
        us = sc.u_src[k, slot]
        if us < sc.base:
            hs16[:, sc.ucol[slot]] = init_val(us).astype(np.float16)
        vs = sc.i_src[k, slot]
        if vs < sc.base:
            hs16[:, sc.vcol[slot]] = init_val(vs).astype(np.float16)

    vbinit = np.zeros((E, sc.ncompact), f)
    for src, c in sc.cmap[k].items():
        vbinit[:, c] = init_val(src)

    gidx = _wrap_idx_combined(sc, k)
    return hs16, vbinit, gidx


# ----------------------------------------------------------------------------
# pure-numpy model of the scheduled computation (validation / debugging)
# ----------------------------------------------------------------------------

def _numpy_model(inp, sc):
    f16 = np.float16
    wstack, bst, bm = _prep_shared(inp)
    ws = wstack.astype(np.float32)
    bs = bst.astype(np.float32)
    ne = sc.ne
    out = np.zeros((sc.nev, 2), np.float32)

    def blk(i):
        return ws[:, i * E:(i + 1) * E]

    for k in range(NCORES):
        hs16, vbinit, _ = _core_inputs(inp, sc, k)
        vb = np.zeros((E, sc.nvcols), np.float32)
        vb[:, :sc.ncompact] = vbinit
        hs = hs16.astype(np.float32)
        dotv = np.zeros(ne, np.float32)
        logitv = np.zeros(ne, np.float32)
        for (l, b, off, vbc, _, un, vn) in sc.steps:
            for j in range(un):
                c = sc.dev_col(k, sc.u_src[k, off + j])
                hs[:, 2 * off + j] = vb[:, c].astype(f16).astype(np.float32)
            for j in range(vn):
                c = sc.dev_col(k, sc.i_src[k, off + j])
                hs[:, 2 * off + b + j] = vb[:, c].astype(f16).astype(np.float32)
            ug = hs[:, 2 * off:2 * off + b]
            vg = hs[:, 2 * off + b:2 * off + 2 * b]
            bsel = np.zeros((E, 2 * b), np.float32)
            pr = np.concatenate([np.tile(bs[0:1, 0:E].T, b),
                                 np.tile(bs[1:2, 0:E].T, b)], 1)
            pr[:, :b] += blk(0).T @ vg + blk(1).T @ ug
            pr[:, b:] += blk(2).T @ ug + blk(3).T @ vg
            pz = np.concatenate([np.tile(bs[0:1, E:2 * E].T, b),
                                 np.tile(bs[1:2, E:2 * E].T, b)], 1)
            pz[:, :b] += blk(4).T @ vg + blk(5).T @ ug
            pz[:, b:] += blk(6).T @ ug + blk(7).T @ vg
            pinn = np.concatenate([np.tile(bs[0:1, 2 * E:3 * E].T, b),
                                   np.tile(bs[1:2, 2 * E:3 * E].T, b)], 1)
            pinn[:, :b] += blk(8).T @ vg
            pinn[:, b:] += blk(9).T @ ug
            phn = np.concatenate([np.tile(bs[0:1, 3 * E:].T, b),
                                  np.tile(bs[1:2, 3 * E:].T, b)], 1)
            phn[:, :b] += blk(10).T @ ug
            phn[:, b:] += blk(11).T @ vg
            r = (1.0 / (1.0 + np.exp(-pr))).astype(f16).astype(np.float32)
            z = (1.0 / (1.0 + np.exp(-pz))).astype(f16).astype(np.float32)
            n = np.tanh(phn * r + pinn).astype(f16).astype(np.float32)
            hcat = np.concatenate([ug, vg], axis=1)
            d = (hcat - n).astype(f16).astype(np.float32)
            m = (z * d).astype(f16).astype(np.float32)
            vb[:, vbc:vbc + 2 * b] = n + m
            # per-step MLP
            t1a = ws[:, 12 * E:13 * E]
            t1b = ws[:, 13 * E:14 * E]
            t2 = ws[:, 14 * E:14 * E + 32]
            t3 = ws[:32, 14 * E + 32]
            h1 = np.maximum(t1a.T @ ug + t1b.T @ vg + bm[:, 0:1], 0.0)
            h1 = h1.astype(f16).astype(np.float32)
            h2 = np.maximum(t2.T @ h1 + bm[:32, 1:2], 0.0)
            h2 = h2.astype(f16).astype(np.float32)
            logitv[off:off + b] = t3 @ h2 + bm[0, 2]
            uvm = (ug * vg).astype(f16).astype(np.float32)
            dotv[off:off + b] = uvm.sum(axis=0)
        mask = sc.gid[k] >= 0
        g = sc.gid[k][mask]
        out[g, 0] = dotv[mask]
        out[g, 1] = logitv[mask]
    return out


# ----------------------------------------------------------------------------
# device program
# ----------------------------------------------------------------------------

def _build_program(sc):
    import concourse.bass as bass
    import concourse.tile as tile
    from concourse import bacc, mybir
    from concourse.tile_rust import add_dep_helper

    f32 = mybir.dt.float32
    f16 = mybir.dt.float16
    i16 = mybir.dt.int16
    ne = sc.ne
    W = 14 * E + 32 + 2    # wstack cols
    W3 = 14 * E + 32       # t3 col
    WON = W3 + 1           # ones col
    AF = mybir.ActivationFunctionType
    OP = mybir.AluOpType

    nc = bacc.Bacc("TRN2", target_bir_lowering=False, debug=False)
    d_w = nc.dram_tensor("wstack", [E, W], f16, kind="ExternalInput").ap()
    d_bs = nc.dram_tensor("bstack", [2, 4 * E], f16, kind="ExternalInput").ap()
    d_bm = nc.dram_tensor("bmisc", [E, 4], f32, kind="ExternalInput").ap()
    d_sel = nc.dram_tensor("sel", [2, 2 * ne], f16, kind="ExternalInput").ap()
    d_hs = nc.dram_tensor("hsinit", [E, 2 * ne], f16, kind="ExternalInput").ap()
    d_gi = nc.dram_tensor("gidx", [E, sc.nicol], i16, kind="ExternalInput").ap()
    d_vb = nc.dram_tensor("vbinit", [E, sc.ncompact], f32, kind="ExternalInput").ap()
    d_out = nc.dram_tensor("outdl", [1, 2 * ne], f32, kind="ExternalOutput").ap()

    b0 = sc.steps[0][1]    # first-step width for the priority DMA slice

    with tile.TileContext(nc) as tc, ExitStack() as ctx:
        const = ctx.enter_context(tc.tile_pool(name="const", bufs=1))
        psum = ctx.enter_context(tc.tile_pool(name="psum", bufs=2, space="PSUM"))
        work = ctx.enter_context(tc.tile_pool(name="work", bufs=2))

        # dummy gather first: pulls the ext-isa GPSIMD library into IRAM
        # while the input DMAs stream in parallel.
        warm = const.tile([E, 16], f32)
        nc.vector.memset(warm[:], 0.0)
        warmi = const.tile([E, 2], i16)
        nc.vector.memset(warmi[:].bitcast(f32), 0.0)
        warmo = const.tile([E, 16], f32)
        nc.gpsimd.ap_gather(warmo[:], warm[:], warmi[:, 0:1],
                            channels=E, num_elems=16, d=1, num_idxs=16)



        # input DMAs, priority order: step-0 working set first
        wsb = const.tile([E, W], f16)
        nc.sync.dma_start(wsb[:, 0:12 * E], d_w[:, 0:12 * E])
        bsb = const.tile([2, 4 * E], f16)
        nc.sync.dma_start(bsb[:], d_bs[:])
        selsb = const.tile([2, 2 * ne], f16)
        nc.sync.dma_start(selsb[:], d_sel[:])
        hs = const.tile([E, 2 * ne], f16)
        nc.sync.dma_start(hs[:, 0:2 * b0], d_hs[:, 0:2 * b0])
        bmsb = const.tile([E, 4], f32)
        nc.sync.dma_start(bmsb[:], d_bm[:])
        nc.sync.dma_start(hs[:, 2 * b0:], d_hs[:, 2 * b0:])
        nc.sync.dma_start(wsb[:, 12 * E:], d_w[:, 12 * E:])
        gidx = const.tile([E, sc.nicol], i16)
        nc.sync.dma_start(gidx[:], d_gi[:])
        vbuf = const.tile([E, sc.nvcols], f32)
        nc.sync.dma_start(vbuf[:, :sc.ncompact], d_vb[:])
        scr = const.tile([E, 64], f32)
        outsb = const.tile([1, 2 * ne], f32)   # [dot | logit]

        def mm(out_ap, wcol, rhs_ap, start, stop):
            nc.tensor.matmul(
                out_ap,
                lhsT=wsb[:, wcol * E:(wcol + 1) * E],
                rhs=rhs_ap,
                start=start, stop=stop, skip_group_check=True,
            )

        def gru_mms(pt4, ug, vg, off, b):
            pr, pz, pinn, phn = pt4
            selb = selsb[:, 2 * off:2 * off + 2 * b]
            for gi, pt in enumerate((pr, pz, pinn, phn)):
                nc.tensor.matmul(
                    pt[:, 0:2 * b], lhsT=bsb[:, gi * E:(gi + 1) * E],
                    rhs=selb, start=True, stop=False, skip_group_check=True)
            # r first (critical path), then hn, inn, z
            mm(pr[:, 0:b], 0, vg, False, False)
            mm(pr[:, 0:b], 1, ug, False, True)
            mm(pr[:, b:2 * b], 2, ug, False, False)
            mm(pr[:, b:2 * b], 3, vg, False, True)
            mm(phn[:, 0:b], 10, ug, False, True)
            mm(phn[:, b:2 * b], 11, vg, False, True)
            mm(pinn[:, 0:b], 8, vg, False, True)
            mm(pinn[:, b:2 * b], 9, ug, False, True)
            mm(pz[:, 0:b], 4, vg, False, False)
            mm(pz[:, 0:b], 5, ug, False, True)
            mm(pz[:, b:2 * b], 6, ug, False, False)
            mm(pz[:, b:2 * b], 7, vg, False, True)

        def gru_tail(pt4, step, wb_list, eng=None):
            (l, b, off, vbc, ic, un, vn) = step
            pr, pz, pinn, phn = pt4
            ve = eng if eng is not None else nc.vector
            r = work.tile([E, 2 * b], f16, tag="r")
            z = work.tile([E, 2 * b], f16, tag="z")
            nfn = work.tile([E, 2 * b], f16, tag="nfn")
            tmp = work.tile([E, 2 * b], f32, tag="tmp")
            d16 = work.tile([E, 2 * b], f16, tag="d16")
            nc.scalar.activation(r[:], pr[:], AF.Sigmoid)
            # PSUM-reading ops must stay on the vector engine
            nc.vector.tensor_tensor(out=tmp[:], in0=phn[:], in1=r[:], op=OP.mult)
            nc.vector.tensor_tensor(out=tmp[:], in0=tmp[:], in1=pinn[:], op=OP.add)
            nc.scalar.activation(nfn[:], tmp[:], AF.Tanh)
            nc.scalar.activation(z[:], pz[:], AF.Sigmoid)
            hcat = hs[:, 2 * off:2 * off + 2 * b]
            ve.tensor_tensor(out=d16[:], in0=hcat, in1=nfn[:], op=OP.subtract)
            ve.tensor_tensor(out=d16[:], in0=z[:], in1=d16[:], op=OP.mult)
            wb = ve.tensor_tensor(
                out=vbuf[:, vbc:vbc + 2 * b],
                in0=nfn[:], in1=d16[:], op=OP.add)
            wb_list.append(wb)

        wb_list = []

        def gather_cast(step):
            (l, b, off, vbc, ic, un, vn) = step
            if un + vn == 0:
                return
            g = nc.gpsimd.ap_gather(
                scr[:, 0:un + vn], vbuf[:], gidx[:, ic:ic + (un + vn) // LANE],
                channels=E, num_elems=sc.nvcols, d=1, num_idxs=un + vn)
            for wb in wb_list[-2:]:
                add_dep_helper(g.ins, wb.ins,
                               reason="gather reads prev writeback")
            if un:
                nc.vector.tensor_copy(out=hs[:, 2 * off:2 * off + un],
                                      in_=scr[:, 0:un])
            if vn:
                nc.vector.tensor_copy(out=hs[:, 2 * off + b:2 * off + b + vn],
                                      in_=scr[:, un:un + vn])

        def mlp_chunk(step, uvm_eng=None):
            (l, b, off, vbc, ic, un, vn) = step
            ue = uvm_eng if uvm_eng is not None else nc.vector
            u_c = hs[:, 2 * off:2 * off + b]
            v_c = hs[:, 2 * off + b:2 * off + 2 * b]
            h1p = psum.tile([E, b], f32, tag="pr", name="h1p")
            mm(h1p[:], 12, u_c, True, False)
            mm(h1p[:], 13, v_c, False, True)
            h1 = work.tile([E, b], f16, tag="r", name="h1")
            nc.scalar.activation(h1[:], h1p[:], AF.Relu, bias=bmsb[:, 0:1])
            h2p = psum.tile([32, b], f32, tag="pz", name="h2p")
            nc.tensor.matmul(h2p[:], lhsT=wsb[:, 14 * E:14 * E + 32],
                             rhs=h1[:], start=True, stop=True,
                             skip_group_check=True)
            h2 = work.tile([32, b], f16, tag="z", name="h2")
            nc.scalar.activation(h2[:], h2p[:], AF.Relu, bias=bmsb[:32, 1:2])
            h3p = psum.tile([1, b], f32, tag="pinn", name="h3p")
            nc.tensor.matmul(h3p[:], lhsT=wsb[:32, W3:W3 + 1],
                             rhs=h2[:], start=True, stop=True,
                             skip_group_check=True)
            nc.scalar.activation(outsb[0:1, ne + off:ne + off + b], h3p[:],
                                 AF.Identity, bias=bmsb[0:1, 2:3])
            uvm = work.tile([E, b], f16, tag="nfn", name="uvm")
            ue.tensor_tensor(out=uvm[:], in0=u_c, in1=v_c, op=OP.mult)
            dotp = psum.tile([1, b], f32, tag="phn", name="dotp")
            nc.tensor.matmul(dotp[:], lhsT=wsb[:, WON:WON + 1],
                             rhs=uvm[:], start=True, stop=True,
                             skip_group_check=True)
            nc.vector.tensor_copy(out=outsb[0:1, off:off + b], in_=dotp[:])

        def psum4(b):
            return tuple(psum.tile([E, 2 * b], f32, tag=t, name=f"p_{t}")
                         for t in ("pr", "pz", "pinn", "phn"))

        big = [st for st in sc.steps if st[5] + st[6] == 0]
        tail = [st for st in sc.steps if st[5] + st[6] > 0]
        assert len(big) <= 2, "level-0 region must fit 2 PSUM generations"

        pts = {}
        for st in big:
            (l, b, off) = st[0], st[1], st[2]
            pt4 = psum4(b)
            pts[off] = pt4
            ug = hs[:, 2 * off:2 * off + b]
            vg = hs[:, 2 * off + b:2 * off + 2 * b]
            gru_mms(pt4, ug, vg, off, b)
        for si, st in enumerate(big):
            gru_tail(pts[st[2]], st, wb_list)

        if tail:
            gather_cast(tail[0])
        for i, st in enumerate(tail):
            (l, b, off) = st[0], st[1], st[2]
            pt4 = psum4(b)
            ug = hs[:, 2 * off:2 * off + b]
            vg = hs[:, 2 * off + b:2 * off + 2 * b]
            gru_mms(pt4, ug, vg, off, b)
            gru_tail(pt4, st, wb_list)
            # issue the next gather immediately after this writeback so the
            # GPSIMD wake latency hides under the MLP backlog below
            if i + 1 < len(tail):
                gather_cast(tail[i + 1])
            if i < len(big):
                mlp_chunk(big[i])
            mlp_chunk(st)
            if i == len(tail) - 1:
                for j in range(len(tail), len(big)):
                    mlp_chunk(big[j])
                # ship the big-region outputs while the tail finishes
                big_end = big[-1][2] + big[-1][1] if big else 0
                if big_end:
                    nc.sync.dma_start(d_out[:, 0:big_end], outsb[:, 0:big_end])
                    nc.sync.dma_start(d_out[:, ne:ne + big_end],
                                      outsb[:, ne:ne + big_end])
        if not tail:
            for st in big:
                mlp_chunk(st)
            big_end = 0

        tail_start = big[-1][2] + big[-1][1] if big else 0
        nc.sync.dma_start(d_out[:, tail_start:ne], outsb[:, tail_start:ne])
        nc.sync.dma_start(d_out[:, ne + tail_start:], outsb[:, ne + tail_start:])

    nc.compile()
    return nc


# ----------------------------------------------------------------------------
# entry point
# ----------------------------------------------------------------------------

def kernel(**inputs):
    global LAST_EXEC_NS
    from concourse.bass_utils import run_bass_kernel_spmd

    uid = np.asarray(inputs["user_ids"])
    iid = np.asarray(inputs["item_ids"])
    key = (uid.tobytes(), iid.tobytes())
    if key not in _CACHE:
        sc = _build_schedule(uid, iid)
        nc = _build_program(sc)
        _CACHE[key] = (sc, nc)
    sc, nc = _CACHE[key]

    wstack, bst, bm = _prep_shared(inputs)
    sel = _sel_array(sc)
    in_maps = []
    for k in range(NCORES):
        hs16, vbinit, gidx = _core_inputs(inputs, sc, k)
        in_maps.append({
            "wstack": wstack, "bstack": bst, "bmisc": bm, "sel": sel,
            "hsinit": hs16, "gidx": gidx, "vbinit": vbinit,
        })

    res = run_bass_kernel_spmd(nc, in_maps, list(range(NCORES)), trace=TRACE)
    LAST_EXEC_NS = res.exec_time_ns

    out = np.zeros((sc.nev, 2), np.float32)
    for k in range(NCORES):
        mask = sc.gid[k] >= 0
        g = sc.gid[k][mask]
        o = res.results[k]["outdl"][0]
        dot = o[:sc.ne][mask].astype(np.float64)
        logit = o[sc.ne:][mask].astype(np.float64)
        sp = np.logaddexp(0.0, dot)
        out[g, 0] = (-np.log(sp + 1e-10)).astype(np.float32)
        out[g, 1] = (1.0 / (1.0 + np.exp(-logit))).astype(np.float32)
    return out
